# revision 1
# baseline (speedup 1.0000x reference)
"""Bass/Tile TRN2 kernel for nn_Decoder (LSTM captioning decoder with spatial
attention). B=128, K=49, D=512, E=256, V=10000, T=20, 8 NeuronCores.

Sharding: recurrence + attention replicated on all 8 cores (batch 128 = full
partition width); the dominant logit matmul (c+h) @ Wp.T sharded over vocab
(1280 padded rows per core). No collectives; host concatenates vocab slices.

State layout is [batch=128 partitions, feature]. Matmuls are
activation-stationary (lhsT = PE-transposed activations, rhs = wide weight
panels) so fp32 runs single-pass as float32r at 1 cycle/row. The attention
context c = sum_k alpha[b,k]*spatial[b,k,:] uses a diag trick: lhsT =
diag(alpha[:,k]) built by a scaled copy of the identity, rhs = spatial[:,k,:],
PSUM-accumulated over the 49 positions.
"""

import sys

for _p in ("/opt/trn_rl_repo", "/opt/pypackages"):
    if _p not in sys.path:
        sys.path.append(_p)

import numpy as np
import ml_dtypes

import concourse.bass as bass
from concourse import bacc
import concourse.mybir as mybir
import concourse.tile as tile
from concourse.bass_utils import run_bass_kernel_spmd

F32 = mybir.dt.float32
F32R = mybir.dt.float32r
BF16 = mybir.dt.bfloat16
AF = mybir.ActivationFunctionType
ALU = mybir.AluOpType

B, K, D, E, V, T = 128, 49, 512, 256, 10000, 20
NCORES = 8
VS = 1280
KT_X = 7           # x-part contraction tiles: 768 data rows + ones row -> 896
KT_H = 4
GD = 4 * D
NPB = np.dtype(ml_dtypes.bfloat16)


def _build_nc():
    nc = bacc.Bacc("TRN2", target_bir_lowering=False, debug=False)

    d_xw = nc.dram_tensor("xw", [T, 128, KT_X * 128], BF16, kind="ExternalInput")
    d_wih = nc.dram_tensor("wih", [128, KT_X, GD], BF16, kind="ExternalInput")
    d_whh = nc.dram_tensor("whh", [128, KT_H, GD], F32R, kind="ExternalInput")
    d_wg = nc.dram_tensor("wg", [128, KT_H, K + 1], F32R, kind="ExternalInput")
    d_wp = nc.dram_tensor("wp", [128, KT_H, VS], F32R, kind="ExternalInput")
    d_spat = nc.dram_tensor("spat", [128, K, D], BF16, kind="ExternalInput")
    d_spatT = nc.dram_tensor("spatT", [128, K, 4, 128], BF16, kind="ExternalInput")
    d_wv = nc.dram_tensor("wv", [128, 4, K], BF16, kind="ExternalInput")
    d_bvbg = nc.dram_tensor("bvbg", [128, K], F32, kind="ExternalInput")
    d_wh = nc.dram_tensor("whv", [128, K], F32, kind="ExternalInput")
    d_bp = nc.dram_tensor("bp", [128, VS], F32, kind="ExternalInput")
    d_mask = nc.dram_tensor("mask", [128, T], F32, kind="ExternalInput")
    d_nmask = nc.dram_tensor("nmask", [128, T], F32, kind="ExternalInput")
    d_idf = nc.dram_tensor("idf", [128, 128], F32, kind="ExternalInput")
    d_idb = nc.dram_tensor("idb", [128, 128], BF16, kind="ExternalInput")
    d_gfT = nc.dram_tensor("gfT", [128, 5, 128], F32R, kind="ExternalInput")
    d_winit = nc.dram_tensor("winit", [128, 5, 2 * D], F32R, kind="ExternalInput")
    d_out = nc.dram_tensor("out", [T, 128, VS], F32, kind="ExternalOutput")

    r = lambda ap: ap.bitcast(F32R)

    with tile.TileContext(nc) as tc:
        with (
            tc.tile_pool(name="const", bufs=1) as cp,
            tc.tile_pool(name="state", bufs=1) as sp,
            tc.tile_pool(name="xwin", bufs=2) as xp,
            tc.tile_pool(name="lstm", bufs=1) as lp,
            tc.tile_pool(name="attw", bufs=1) as ap_,
            tc.tile_pool(name="small", bufs=2) as smp,
            tc.tile_pool(name="lwork", bufs=2) as lwp,
            tc.tile_pool(name="vpin", bufs=2) as vip,
            tc.tile_pool(name="ps_g", bufs=2, space="PSUM") as pg,
            tc.tile_pool(name="ps_l", bufs=2, space="PSUM") as pl,
            tc.tile_pool(name="ps_t", bufs=2, space="PSUM") as pt,
            tc.tile_pool(name="ps_c", bufs=1, space="PSUM") as pc,
        ):
            # ---- resident constants ----
            wih_sb = cp.tile([128, KT_X * GD], BF16, tag="wih")
            nc.sync.dma_start(out=wih_sb[:], in_=d_wih[:].rearrange("p a b -> p (a b)"))
            whh_sb = cp.tile([128, KT_H * GD], F32R, tag="whh")
            nc.sync.dma_start(out=whh_sb[:], in_=d_whh[:].rearrange("p a b -> p (a b)"))
            wg_sb = cp.tile([128, KT_H * (K + 1)], F32R, tag="wg")
            nc.sync.dma_start(out=wg_sb[:], in_=d_wg[:].rearrange("p a b -> p (a b)"))
            wp_sb = cp.tile([128, KT_H * VS], F32R, tag="wp")
            nc.sync.dma_start(out=wp_sb[:], in_=d_wp[:].rearrange("p a b -> p (a b)"))
            spat_sb = cp.tile([128, K * D], BF16, tag="spat")
            nc.sync.dma_start(out=spat_sb[:], in_=d_spat[:].rearrange("p a b -> p (a b)"))
            bvbg_sb = cp.tile([128, K], F32, tag="bvbg")
            nc.sync.dma_start(out=bvbg_sb[:], in_=d_bvbg[:])
            wh_sb = cp.tile([128, K], F32, tag="wh")
            nc.sync.dma_start(out=wh_sb[:], in_=d_wh[:])
            bp_sb = cp.tile([128, VS], F32, tag="bp")
            nc.sync.dma_start(out=bp_sb[:], in_=d_bp[:])
            mask_sb = cp.tile([128, T], F32, tag="mask")
            nc.sync.dma_start(out=mask_sb[:], in_=d_mask[:])
            nmask_sb = cp.tile([128, T], F32, tag="nmask")
            nc.sync.dma_start(out=nmask_sb[:], in_=d_nmask[:])
            idf_sb = cp.tile([128, 128], F32, tag="idf")
            nc.sync.dma_start(out=idf_sb[:], in_=d_idf[:])
            idb_sb = cp.tile([128, 128], BF16, tag="idb")
            nc.sync.dma_start(out=idb_sb[:], in_=d_idb[:])
            vp_sb = cp.tile([128, K * K], F32, tag="vp")
            wv_sb = cp.tile([128, 4 * K], BF16, tag="wv")
            nc.sync.dma_start(out=wv_sb[:], in_=d_wv[:].rearrange("p a b -> p (a b)"))

            # ---- state ----
            h_sb = sp.tile([128, D], F32, tag="h")
            m_sb = sp.tile([128, D], F32, tag="m")
            hT_sb = sp.tile([128, D], F32R, tag="hT")

            # ---- precompute V_proj[b,(k,j)] = spatial @ Wv.T + (bv+bg) ----
            for k in range(K):
                spt = vip.tile([128, 4 * 128], BF16, tag="spt")
                nc.sync.dma_start(
                    out=spt[:], in_=d_spatT[:, k].rearrange("p a b -> p (a b)")
                )
                ps = pt.tile([128, 512], F32, tag="tr")
                for dt_ in range(4):
                    nc.tensor.matmul(
                        ps[:, :K],
                        spt[:, dt_ * 128 : (dt_ + 1) * 128],
                        wv_sb[:, dt_ * K : (dt_ + 1) * K],
                        start=(dt_ == 0),
                        stop=(dt_ == 3),
                    )
                nc.vector.tensor_add(vp_sb[:, k * K : (k + 1) * K], ps[:, :K], bvbg_sb[:])

            # ---- init h0/m0 = gf @ W_init.T + b_init (bias via ones row) ----
            gfT_sb = vip.tile([128, 5 * 128], F32R, tag="gfT")
            nc.sync.dma_start(out=gfT_sb[:], in_=d_gfT[:].rearrange("p a b -> p (a b)"))
            for which, dst in ((0, h_sb), (1, m_sb)):
                for ch in range(2):
                    ps = pt.tile([128, 512], F32, tag="tr")
                    for kt in range(5):
                        wr = vip.tile([128, 256], F32R, tag="wr")
                        nc.sync.dma_start(
                            out=wr[:],
                            in_=d_winit[:, kt, which * D + ch * 256 : which * D + ch * 256 + 256],
                        )
                        nc.tensor.matmul(
                            ps[:, :256],
                            gfT_sb[:, kt * 128 : (kt + 1) * 128],
                            wr[:],
                            start=(kt == 0),
                            stop=(kt == 4),
                        )
                    nc.scalar.activation(
                        dst[:, ch * 256 : (ch + 1) * 256], ps[:, :256], AF.Copy
                    )

            def transpose_to(dst_sb, src_sb):
                for dt_ in range(4):
                    pst = pt.tile([128, 512], F32, tag="tr")
                    nc.tensor.transpose(
                        pst[:, :128], src_sb[:, dt_ * 128 : (dt_ + 1) * 128], idf_sb[:]
                    )
                    nc.vector.tensor_copy(
                        dst_sb[:, dt_ * 128 : (dt_ + 1) * 128], pst[:, :128]
                    )

            transpose_to(hT_sb, h_sb)

            def emit_gates_and_evict(t, xw_tile):
                # returns (i,f,g,o) SBUF tiles for step t
                outs = []
                funcs = (AF.Sigmoid, AF.Sigmoid, AF.Tanh, AF.Sigmoid)
                names = ("i", "f", "g", "o")
                for nch in range(4):
                    ps = pg.tile([128, 512], F32, tag="g")
                    for kt in range(KT_X):
                        nc.tensor.matmul(
                            ps[:],
                            xw_tile[:, kt * 128 : (kt + 1) * 128],
                            wih_sb[:, kt * GD + nch * 512 : kt * GD + nch * 512 + 512],
                            start=(kt == 0),
                            stop=False,
                        )
                    for kt in range(KT_H):
                        nc.tensor.matmul(
                            ps[:],
                            hT_sb[:, kt * 128 : (kt + 1) * 128],
                            whh_sb[:, kt * GD + nch * 512 : kt * GD + nch * 512 + 512],
                            start=False,
                            stop=(kt == KT_H - 1),
                        )
                    o = lp.tile([128, D], F32, tag=names[nch])
                    nc.scalar.activation(o[:], ps[:], funcs[nch])
                    outs.append(o)
                return outs

            def emit_state_update(t, ifgo):
                i_sb, f_sb, g_sb, o_sb = ifgo
                mk = mask_sb[:, t : t + 1]
                nmk = nmask_sb[:, t : t + 1]
                # masked update folded into gate scaling:
                # m' = (mask*f + (1-mask)) * m + (mask*i) * g ; h' = (1-mask)*h + (mask*o)*tanh(m')
                t1 = lp.tile([128, D], F32, tag="t1")
                nc.vector.tensor_scalar(i_sb[:], i_sb[:], mk, None, ALU.mult)
                nc.vector.tensor_scalar(f_sb[:], f_sb[:], mk, nmk, ALU.mult, ALU.add)
                nc.vector.tensor_mul(t1[:], i_sb[:], g_sb[:])
                nc.vector.tensor_mul(m_sb[:], f_sb[:], m_sb[:])
                nc.vector.tensor_add(m_sb[:], m_sb[:], t1[:])
                tm = lp.tile([128, D], F32, tag="t2")
                nc.scalar.activation(tm[:], m_sb[:], AF.Tanh)
                nc.vector.tensor_scalar(o_sb[:], o_sb[:], mk, None, ALU.mult)
                nc.vector.tensor_mul(tm[:], o_sb[:], tm[:])
                nc.vector.tensor_scalar(h_sb[:], h_sb[:], nmk, None, ALU.mult)
                nc.vector.tensor_add(h_sb[:], h_sb[:], tm[:])
                transpose_to(hT_sb, h_sb)

            def emit_logits(t, chT):
                for c0, cw in ((0, 512), (512, 512), (1024, 256)):
                    ps = pl.tile([128, 512], F32, tag="l")
                    for kt in range(KT_H):
                        nc.tensor.matmul(
                            ps[:, :cw],
                            chT[:, kt * 128 : (kt + 1) * 128],
                            wp_sb[:, kt * VS + c0 : kt * VS + c0 + cw],
                            start=(kt == 0),
                            stop=(kt == KT_H - 1),
                        )
                    lo = lwp.tile([128, 512], F32, tag="lo")
                    nc.vector.tensor_add(lo[:, :cw], ps[:, :cw], bp_sb[:, c0 : c0 + cw])
                    nc.vector.tensor_scalar_mul(
                        lo[:, :cw], lo[:, :cw], mask_sb[:, t : t + 1]
                    )
                    nc.sync.dma_start(out=d_out[t, :, c0 : c0 + cw], in_=lo[:, :cw])

            xw0 = xp.tile([128, KT_X * 128], BF16, tag="xw")
            nc.sync.dma_start(out=xw0[:], in_=d_xw[0])
            ifgo = emit_gates_and_evict(0, xw0)
            emit_state_update(0, ifgo)

            for t in range(T):
                # ---- attention on h_t ----
                hg_ps = pt.tile([128, 512], F32, tag="tr")
                for kt in range(KT_H):
                    nc.tensor.matmul(
                        hg_ps[:, : K + 1],
                        hT_sb[:, kt * 128 : (kt + 1) * 128],
                        wg_sb[:, kt * (K + 1) : (kt + 1) * (K + 1)],
                        start=(kt == 0),
                        stop=(kt == KT_H - 1),
                    )
                hg_sb = smp.tile([128, K], F32, tag="hg")
                nc.scalar.activation(hg_sb[:], hg_ps[:, :K], AF.Copy)

                att = ap_.tile([128, K * K], F32, tag="att")
                att3 = att[:].rearrange("p (k j) -> p k j", j=K)
                nc.vector.tensor_add(
                    att3,
                    vp_sb[:].rearrange("p (k j) -> p k j", j=K),
                    hg_sb[:].unsqueeze(1).broadcast_to([128, K, K]),
                )
                nc.scalar.activation(att[:], att[:], AF.Tanh)
                nc.vector.tensor_mul(
                    att3, att3, wh_sb[:].unsqueeze(1).broadcast_to([128, K, K])
                )
                z_sb = smp.tile([128, K], F32, tag="z")
                nc.vector.tensor_reduce(
                    z_sb[:], att3, axis=mybir.AxisListType.X, op=ALU.add
                )
                zmax = smp.tile([128, 1], F32, tag="zmax")
                nc.vector.tensor_reduce(
                    zmax[:], z_sb[:], axis=mybir.AxisListType.X, op=ALU.max
                )
                nc.vector.tensor_scalar_sub(z_sb[:], z_sb[:], zmax[:])
                alpha = smp.tile([128, K], F32, tag="alpha")
                zsum = smp.tile([128, 1], F32, tag="zsum")
                nc.scalar.activation(alpha[:], z_sb[:], AF.Exp, accum_out=zsum[:])
                zinv = smp.tile([128, 1], F32, tag="zinv")
                nc.vector.reciprocal(zinv[:], zsum[:])
                nc.vector.tensor_scalar_mul(alpha[:], alpha[:], zinv[:])

                # ---- keep PE busy during attention: gates t+1 ----
                if t + 1 < T:
                    xwt = xp.tile([128, KT_X * 128], BF16, tag="xw")
                    nc.sync.dma_start(out=xwt[:], in_=d_xw[t + 1])
                    ifgo = emit_gates_and_evict(t + 1, xwt)

                # ---- logits of the previous step fill the alpha-wait gap ----
                if t > 0:
                    emit_logits(t - 1, prev_chT)

                # ---- context c via diag trick ----
                c_ps = pc.tile([128, D], F32, tag="c")
                for k in range(K):
                    dg = lwp.tile([128, 128], BF16, tag="dg")
                    nc.scalar.activation(
                        dg[:], idb_sb[:], AF.Copy, scale=alpha[:, k : k + 1]
                    )
                    nc.tensor.matmul(
                        c_ps[:],
                        dg[:],
                        spat_sb[:, k * D : (k + 1) * D],
                        start=(k == 0),
                        stop=(k == K - 1),
                    )
                ch_sb = lwp.tile([128, D], F32, tag="ch")
                nc.vector.tensor_add(ch_sb[:], c_ps[:], h_sb[:])
                chT_sb = lwp.tile([128, D], F32R, tag="chT")
                transpose_to(chT_sb, ch_sb)
                prev_chT = chT_sb

                if t + 1 < T:
                    emit_state_update(t + 1, ifgo)

            emit_logits(T - 1, prev_chT)

    nc.compile()
    return nc


_CACHE = {}


def _prep_maps(spatial, global_feats, captions, lengths, emb,
               W_init_h, b_init_h, W_init_m, b_init_m,
               W_ih, b_ih, W_hh, b_hh, Wv, bv, Wg, bg, wh, bh_att, Wp, bp):
    f32 = np.float32
    spatial = np.asarray(spatial, f32)
    global_feats = np.asarray(global_feats, f32)
    captions = np.asarray(captions)
    lengths = np.asarray(lengths)
    emb = np.asarray(emb, f32)

    emb_seq = emb[captions]                      # [B, T, E]
    xw = np.zeros((T, KT_X * 128, B), f32)
    for t in range(T):
        xw[t, :E] = emb_seq[:, t, :].T
        xw[t, E : E + D] = global_feats.T
        xw[t, E + D] = 1.0
    xw_t = np.ascontiguousarray(
        xw.reshape(T, KT_X, 128, B).transpose(0, 2, 1, 3).reshape(T, 128, KT_X * B)
    ).astype(NPB)

    wcat = np.concatenate([np.asarray(W_ih, f32).T,
                           (np.asarray(b_ih, f32) + np.asarray(b_hh, f32))[None, :]], 0)
    wihx = np.zeros((KT_X * 128, GD), f32)
    wihx[: wcat.shape[0]] = wcat
    wih = np.ascontiguousarray(wihx.reshape(KT_X, 128, GD).transpose(1, 0, 2)).astype(NPB)

    whh = np.ascontiguousarray(
        np.asarray(W_hh, f32).T.reshape(KT_H, 128, GD).transpose(1, 0, 2))
    wgp = np.zeros((D, K + 1), f32)
    wgp[:, :K] = np.asarray(Wg, f32).T
    wg = np.ascontiguousarray(wgp.reshape(KT_H, 128, K + 1).transpose(1, 0, 2))

    spat_b = spatial.astype(NPB)
    spatT = np.ascontiguousarray(
        spatial.transpose(1, 2, 0).reshape(K, 4, 128, B).transpose(2, 0, 1, 3)
    ).astype(NPB)
    wv = np.ascontiguousarray(
        np.asarray(Wv, f32).T.reshape(4, 128, K).transpose(1, 0, 2)).astype(NPB)

    bvbg = np.broadcast_to(
        (np.asarray(bv, f32) + np.asarray(bg, f32)), (128, K)).copy()
    whv = np.broadcast_to(np.asarray(wh, f32)[0], (128, K)).copy()
    maskf = (np.arange(T)[None, :] < np.asarray(lengths)[:, None]).astype(f32)
    nmaskf = (1.0 - maskf).astype(f32)
    idf = np.eye(128, dtype=f32)
    idb = np.eye(128).astype(NPB)

    gfT = np.zeros((5, 128, B), f32)
    gfT[:4] = global_feats.T.reshape(4, 128, B)
    gfT[4, 0] = 1.0
    gfT = np.ascontiguousarray(gfT.transpose(1, 0, 2))

    winit = np.zeros((5 * 128, 2 * D), f32)
    winit[:D, :D] = np.asarray(W_init_h, f32).T
    winit[:D, D:] = np.asarray(W_init_m, f32).T
    winit[D] = np.concatenate([np.asarray(b_init_h, f32), np.asarray(b_init_m, f32)])
    winit = np.ascontiguousarray(winit.reshape(5, 128, 2 * D).transpose(1, 0, 2))

    common = dict(
        xw=xw_t, wih=wih, whh=whh, wg=wg, spat=spat_b, spatT=spatT, wv=wv,
        bvbg=bvbg, whv=whv, mask=maskf, nmask=nmaskf, idf=idf, idb=idb, gfT=gfT, winit=winit,
    )

    in_maps = []
    Wp = np.asarray(Wp, f32)
    bp = np.asarray(bp, f32)
    for c in range(NCORES):
        lo = VS * c
        wps = np.zeros((VS, D), f32)
        bps = np.zeros((VS,), f32)
        n = max(0, min(VS, V - lo))
        if n:
            wps[:n] = Wp[lo : lo + n]
            bps[:n] = bp[lo : lo + n]
        wpt = np.ascontiguousarray(wps.T.reshape(KT_H, 128, VS).transpose(1, 0, 2))
        bpt = np.broadcast_to(bps, (128, VS)).copy()
        in_maps.append(dict(common, wp=wpt, bp=bpt))
    return in_maps


def kernel(**inputs):
    in_maps = _prep_maps(**inputs)
    if "nc" not in _CACHE:
        _CACHE["nc"] = _build_nc()
    res = run_bass_kernel_spmd(_CACHE["nc"], in_maps, list(range(NCORES)))
    logits = np.empty((B, T, V), np.float32)
    for c in range(NCORES):
        lo = VS * c
        n = max(0, min(VS, V - lo))
        if n:
            oc = np.asarray(res.results[c]["out"])
            logits[:, :, lo : lo + n] = oc[:, :, :n].transpose(1, 0, 2)
    return logits

